# revision 10
# baseline (speedup 1.0000x reference)
"""LIF spike kernel for Trainium2 (8 NeuronCores, batch-parallel).

Problem: x [64,128,56,56] f32; LIF recurrence in blocks of lif=8 steps along
H (dim=2):
    u   = tau*o + x_j
    out = u if u > vth else vth          == max(u, vth)
    o'  = 0 if u > vth else u            == (u <= vth) * u

Sharding: batch dim 64 -> 8 cores x 8 batches, no communication.
Per-core layout: C=128 on partitions, free dim = H*W per batch tile.
"""

import numpy as np

_B, _C, _H, _W = 64, 128, 56, 56
_NCORES = 8
_BS = _B // _NCORES          # batches per core
_LIF = 8
_NB = _H // _LIF             # 7 blocks per image

_CACHE = {}


def _build(tau: float, vth: float):
    import concourse.bacc as bacc
    import concourse.mybir as mybir
    from concourse.tile import TileContext

    f32 = mybir.dt.float32
    op = mybir.AluOpType

    # Bacc (not raw Bass): its compile() runs generate_event_semaphores,
    # which splits multi-waits into EventSemaphore instructions (TRN2 allows
    # only one sem wait per instruction).
    nc = bacc.Bacc("TRN2")
    x = nc.dram_tensor("x", [_BS, _C, _H, _W], f32, kind="ExternalInput")
    y = nc.dram_tensor("y", [_BS, _C, _H, _W], f32, kind="ExternalOutput")

    HW = _H * _W

    # Engine split per step j (recurrence along the lif axis):
    #   DVE:    u  = (o * tau) + x_j        [scalar_tensor_tensor]
    #           o' = (u <= vth) * u         [scalar_tensor_tensor]
    #   GPSIMD: out_j = max(u, vth)         [1-input op, ~line rate]
    # DMA-ins on the SP HWDGE ring, DMA-outs on the ACT ring so the two
    # streams don't serialize behind one descriptor FIFO.
    FD = _NB * _W                # per-step free-dim elements (392)
    with TileContext(nc) as tc:
        with (
            tc.tile_pool(name="xin", bufs=4) as xp,
            tc.tile_pool(name="out", bufs=4) as outp,
            tc.tile_pool(name="tmp", bufs=3) as tp,
        ):
            for b in range(_BS):
                xt = xp.tile([_C, HW], f32, tag="xt")
                nc.sync.dma_start(out=xt[:, :], in_=x[b].rearrange("c h w -> c (h w)"))
                ot = outp.tile([_C, HW], f32, tag="ot")
                u = tp.tile([_C, FD], f32, tag="u")
                o = tp.tile([_C, FD], f32, tag="o")

                # (nb) blocks are independent; j indexes within each block.
                xv = xt[:, :].rearrange("c (q l w) -> c q l w", l=_LIF, w=_W)
                ov = ot[:, :].rearrange("c (q l w) -> c q l w", l=_LIF, w=_W)
                u3 = u[:, :].rearrange("c (q w) -> c q w", w=_W)
                o3 = o[:, :].rearrange("c (q w) -> c q w", w=_W)

                for j in range(_LIF):
                    xj = xv[:, :, j, :]
                    if j == 0:
                        uin = xj                       # u_0 = x_0 (o starts at 0)
                    else:
                        # u = (o * tau) + x_j
                        nc.vector.scalar_tensor_tensor(
                            out=u3, in0=o3, scalar=tau, in1=xj,
                            op0=op.mult, op1=op.add,
                        )
                        uin = u3
                    if j < _LIF - 1:
                        # o' = (u <= vth) * u
                        nc.vector.scalar_tensor_tensor(
                            out=o3, in0=uin, scalar=vth, in1=uin,
                            op0=op.is_le, op1=op.mult,
                        )
                    # out_j = max(u, vth)
                    nc.gpsimd.tensor_scalar_max(out=ov[:, :, j, :], in0=uin, scalar1=vth)
                nc.scalar.dma_start(
                    out=y[b].rearrange("c h w -> c (h w)"), in_=ot[:, :],
                )
    nc.finalize()   # runs Bacc.compile(): reg alloc + event-sem wait splitting
    return nc


def _get_nc(tau: float, vth: float):
    key = (tau, vth)
    if key not in _CACHE:
        _CACHE[key] = _build(tau, vth)
    return _CACHE[key]


def _run(x, tau, vth, **spmd_kwargs):
    from concourse.bass_utils import run_bass_kernel_spmd

    x = np.ascontiguousarray(np.asarray(x, dtype=np.float32))
    assert x.shape == (_B, _C, _H, _W), x.shape
    tau_f = float(np.asarray(tau).reshape(-1)[0])
    vth_f = float(np.asarray(vth).reshape(-1)[0])
    nc = _get_nc(tau_f, vth_f)
    shards = np.split(x, _NCORES, axis=0)
    in_maps = [{"x": np.ascontiguousarray(s)} for s in shards]
    res = run_bass_kernel_spmd(nc, in_maps, list(range(_NCORES)), **spmd_kwargs)
    out = np.concatenate([r["y"] for r in res.results], axis=0)
    return out, res


def kernel(x, tau, vth, lif, dim):
    assert int(np.asarray(lif)) == _LIF and int(np.asarray(dim)) == 2
    out, _ = _run(x, tau, vth)
    return out


# revision 12
# speedup vs baseline: 3.7762x; 3.7762x over previous
"""LIF spike kernel for Trainium2 (8 NeuronCores, batch-parallel).

Problem: x [64,128,56,56] f32; LIF recurrence in blocks of lif=8 steps along
H (dim=2):
    u   = tau*o + x_j
    out = u if u > vth else vth          == max(u, vth)
    o'  = 0 if u > vth else u            == (u <= vth) * u

Sharding: batch dim 64 -> 8 cores x 8 batches, no communication.
Per-core layout: C=128 on partitions, free dim = H*W per batch tile.
"""

import numpy as np

_B, _C, _H, _W = 64, 128, 56, 56
_NCORES = 8
_BS = _B // _NCORES          # batches per core
_LIF = 8
_NB = _H // _LIF             # 7 blocks per image

_CACHE = {}


def _build(tau: float, vth: float):
    import concourse.bacc as bacc
    import concourse.mybir as mybir
    from concourse.tile import TileContext

    f32 = mybir.dt.float32
    op = mybir.AluOpType

    # Bacc (not raw Bass): its compile() runs generate_event_semaphores,
    # which splits multi-waits into EventSemaphore instructions (TRN2 allows
    # only one sem wait per instruction).
    nc = bacc.Bacc("TRN2")
    x = nc.dram_tensor("x", [_BS, _C, _H, _W], f32, kind="ExternalInput")
    y = nc.dram_tensor("y", [_BS, _C, _H, _W], f32, kind="ExternalOutput")

    HW = _H * _W

    # Engine split per step j (recurrence along the lif axis):
    #   DVE:    u  = (o * tau) + x_j        [scalar_tensor_tensor]
    #           o' = (u <= vth) * u         [scalar_tensor_tensor]
    #   GPSIMD: out_j = max(u, vth)         [1-input op, ~line rate]
    # DMA-ins on the SP HWDGE ring, DMA-outs on the ACT ring so the two
    # streams don't serialize behind one descriptor FIFO.
    FD = _NB * _W                # per-step free-dim elements (392)
    with TileContext(nc) as tc:
        with (
            tc.tile_pool(name="xin", bufs=5) as xp,
            tc.tile_pool(name="out", bufs=5) as outp,
            tc.tile_pool(name="tmp", bufs=3) as tp,
        ):
            for b in range(_BS):
                xt = xp.tile([_C, HW], f32, tag="xt")
                nc.sync.dma_start(out=xt[:, :], in_=x[b].rearrange("c h w -> c (h w)"))
                ot = outp.tile([_C, HW], f32, tag="ot")
                u = tp.tile([_C, FD], f32, tag="u")
                o = tp.tile([_C, FD], f32, tag="o")

                # (nb) blocks are independent; j indexes within each block.
                xv = xt[:, :].rearrange("c (q l w) -> c q l w", l=_LIF, w=_W)
                ov = ot[:, :].rearrange("c (q l w) -> c q l w", l=_LIF, w=_W)
                u3 = u[:, :].rearrange("c (q w) -> c q w", w=_W)
                o3 = o[:, :].rearrange("c (q w) -> c q w", w=_W)

                for j in range(_LIF):
                    xj = xv[:, :, j, :]
                    if j == 0:
                        uin = xj                       # u_0 = x_0 (o starts at 0)
                    else:
                        # u = (o * tau) + x_j
                        nc.vector.scalar_tensor_tensor(
                            out=u3, in0=o3, scalar=tau, in1=xj,
                            op0=op.mult, op1=op.add,
                        )
                        uin = u3
                    if j < _LIF - 1:
                        # o' = (u <= vth) * u
                        nc.vector.scalar_tensor_tensor(
                            out=o3, in0=uin, scalar=vth, in1=uin,
                            op0=op.is_le, op1=op.mult,
                        )
                    # out_j = max(u, vth) -- DVE tensor_scalar runs at 2x;
                    # gpsimd per-op overhead made this 4x slower overall.
                    nc.vector.tensor_scalar_max(out=ov[:, :, j, :], in0=uin, scalar1=vth)
                nc.scalar.dma_start(
                    out=y[b].rearrange("c h w -> c (h w)"), in_=ot[:, :],
                )
    nc.finalize()   # runs Bacc.compile(): reg alloc + event-sem wait splitting
    return nc


def _get_nc(tau: float, vth: float):
    key = (tau, vth)
    if key not in _CACHE:
        _CACHE[key] = _build(tau, vth)
    return _CACHE[key]


def _run(x, tau, vth, **spmd_kwargs):
    from concourse.bass_utils import run_bass_kernel_spmd

    x = np.ascontiguousarray(np.asarray(x, dtype=np.float32))
    assert x.shape == (_B, _C, _H, _W), x.shape
    tau_f = float(np.asarray(tau).reshape(-1)[0])
    vth_f = float(np.asarray(vth).reshape(-1)[0])
    nc = _get_nc(tau_f, vth_f)
    shards = np.split(x, _NCORES, axis=0)
    in_maps = [{"x": np.ascontiguousarray(s)} for s in shards]
    res = run_bass_kernel_spmd(nc, in_maps, list(range(_NCORES)), **spmd_kwargs)
    out = np.concatenate([r["y"] for r in res.results], axis=0)
    return out, res


def kernel(x, tau, vth, lif, dim):
    assert int(np.asarray(lif)) == _LIF and int(np.asarray(dim)) == 2
    out, _ = _run(x, tau, vth)
    return out


# revision 14
# speedup vs baseline: 4.1612x; 1.1020x over previous
"""LIF spike kernel for Trainium2 (8 NeuronCores, batch-parallel).

Problem: x [64,128,56,56] f32; LIF recurrence in blocks of lif=8 steps along
H (dim=2):
    u   = tau*o + x_j
    out = u if u > vth else vth          == max(u, vth)
    o'  = 0 if u > vth else u            == (u <= vth) * u

Sharding: batch dim 64 -> 8 cores x 8 batches, no communication.
Per-core layout: C=128 on partitions, free dim = H*W per batch tile.
"""

import numpy as np

_B, _C, _H, _W = 64, 128, 56, 56
_NCORES = 8
_BS = _B // _NCORES          # batches per core
_LIF = 8
_NB = _H // _LIF             # 7 blocks per image

_CACHE = {}


def _build(tau: float, vth: float):
    import concourse.bacc as bacc
    import concourse.mybir as mybir
    from concourse.tile import TileContext

    f32 = mybir.dt.float32
    op = mybir.AluOpType

    # Bacc (not raw Bass): its compile() runs generate_event_semaphores,
    # which splits multi-waits into EventSemaphore instructions (TRN2 allows
    # only one sem wait per instruction).
    nc = bacc.Bacc("TRN2")
    x = nc.dram_tensor("x", [_BS, _C, _H, _W], f32, kind="ExternalInput")
    y = nc.dram_tensor("y", [_BS, _C, _H, _W], f32, kind="ExternalOutput")

    HW = _H * _W

    # Engine split per step j (recurrence along the lif axis):
    #   DVE:    u  = (o * tau) + x_j        [scalar_tensor_tensor]
    #           o' = (u <= vth) * u         [scalar_tensor_tensor]
    #   GPSIMD: out_j = max(u, vth)         [1-input op, ~line rate]
    # DMA-ins on the SP HWDGE ring, DMA-outs on the ACT ring so the two
    # streams don't serialize behind one descriptor FIFO.
    BP = 2                       # batches per tile: amortizes DVE op overhead
    NS = _BS // BP
    FD = BP * _NB * _W           # per-step free-dim elements (784)
    with TileContext(nc) as tc:
        with (
            tc.tile_pool(name="xin", bufs=3) as xp,
            tc.tile_pool(name="out", bufs=3) as outp,
            tc.tile_pool(name="tmp", bufs=2) as tp,
        ):
            for s in range(NS):
                xt = xp.tile([_C, BP * HW], f32, tag="xt")
                nc.sync.dma_start(
                    out=xt[:, :].rearrange("c (b hw) -> c b hw", hw=HW),
                    in_=x[s * BP:(s + 1) * BP].rearrange("b c h w -> c b (h w)"),
                )
                ot = outp.tile([_C, BP * HW], f32, tag="ot")
                u = tp.tile([_C, FD], f32, tag="u")
                o = tp.tile([_C, FD], f32, tag="o")

                # (b nb) merge is valid: b stride = NB*LIF*W, nb stride =
                # LIF*W -> uniform [count BP*NB, stride LIF*W] + j*W offset.
                xv = xt[:, :].rearrange("c (q l w) -> c q l w", l=_LIF, w=_W)
                ov = ot[:, :].rearrange("c (q l w) -> c q l w", l=_LIF, w=_W)
                u3 = u[:, :].rearrange("c (q w) -> c q w", w=_W)
                o3 = o[:, :].rearrange("c (q w) -> c q w", w=_W)

                for j in range(_LIF):
                    xj = xv[:, :, j, :]
                    if j == 0:
                        uin = xj                       # u_0 = x_0 (o starts at 0)
                    else:
                        # u = (o * tau) + x_j
                        nc.vector.scalar_tensor_tensor(
                            out=u3, in0=o3, scalar=tau, in1=xj,
                            op0=op.mult, op1=op.add,
                        )
                        uin = u3
                    if j < _LIF - 1:
                        # o' = (u <= vth) * u
                        nc.vector.scalar_tensor_tensor(
                            out=o3, in0=uin, scalar=vth, in1=uin,
                            op0=op.is_le, op1=op.mult,
                        )
                    # out_j = max(u, vth) -- DVE tensor_scalar runs at 2x;
                    # gpsimd per-op overhead made this 4x slower overall.
                    nc.vector.tensor_scalar_max(out=ov[:, :, j, :], in0=uin, scalar1=vth)
                nc.scalar.dma_start(
                    out=y[s * BP:(s + 1) * BP].rearrange("b c h w -> c b (h w)"),
                    in_=ot[:, :].rearrange("c (b hw) -> c b hw", hw=HW),
                )
    nc.finalize()   # runs Bacc.compile(): reg alloc + event-sem wait splitting
    return nc


def _get_nc(tau: float, vth: float):
    key = (tau, vth)
    if key not in _CACHE:
        _CACHE[key] = _build(tau, vth)
    return _CACHE[key]


def _run(x, tau, vth, **spmd_kwargs):
    from concourse.bass_utils import run_bass_kernel_spmd

    x = np.ascontiguousarray(np.asarray(x, dtype=np.float32))
    assert x.shape == (_B, _C, _H, _W), x.shape
    tau_f = float(np.asarray(tau).reshape(-1)[0])
    vth_f = float(np.asarray(vth).reshape(-1)[0])
    nc = _get_nc(tau_f, vth_f)
    shards = np.split(x, _NCORES, axis=0)
    in_maps = [{"x": np.ascontiguousarray(s)} for s in shards]
    res = run_bass_kernel_spmd(nc, in_maps, list(range(_NCORES)), **spmd_kwargs)
    out = np.concatenate([r["y"] for r in res.results], axis=0)
    return out, res


def kernel(x, tau, vth, lif, dim):
    assert int(np.asarray(lif)) == _LIF and int(np.asarray(dim)) == 2
    out, _ = _run(x, tau, vth)
    return out


# revision 16
# speedup vs baseline: 4.1685x; 1.0017x over previous
"""LIF spike kernel for Trainium2 (8 NeuronCores, batch-parallel).

Problem: x [64,128,56,56] f32; LIF recurrence in blocks of lif=8 steps along
H (dim=2):
    u   = tau*o + x_j
    out = u if u > vth else vth          == max(u, vth)
    o'  = 0 if u > vth else u            == (u <= vth) * u

Sharding: batch dim 64 -> 8 cores x 8 batches, no communication.
Per-core layout: C=128 on partitions, free dim = H*W per batch tile.
"""

import numpy as np

_B, _C, _H, _W = 64, 128, 56, 56
_NCORES = 8
_BS = _B // _NCORES          # batches per core
_LIF = 8
_NB = _H // _LIF             # 7 blocks per image

_CACHE = {}


def _build(tau: float, vth: float):
    import concourse.bacc as bacc
    import concourse.mybir as mybir
    from concourse.tile import TileContext

    f32 = mybir.dt.float32
    op = mybir.AluOpType

    # Bacc (not raw Bass): its compile() runs generate_event_semaphores,
    # which splits multi-waits into EventSemaphore instructions (TRN2 allows
    # only one sem wait per instruction).
    nc = bacc.Bacc("TRN2")
    x = nc.dram_tensor("x", [_BS, _C, _H, _W], f32, kind="ExternalInput")
    y = nc.dram_tensor("y", [_BS, _C, _H, _W], f32, kind="ExternalOutput")

    HW = _H * _W

    # Engine split per step j (recurrence along the lif axis):
    #   DVE:    u  = (o * tau) + x_j        [scalar_tensor_tensor]
    #           o' = (u <= vth) * u         [scalar_tensor_tensor]
    #   GPSIMD: out_j = max(u, vth)         [1-input op, ~line rate]
    # DMA-ins on the SP HWDGE ring, DMA-outs on the ACT ring so the two
    # streams don't serialize behind one descriptor FIFO.
    # Register const bias APs for the ScalarE activation path (+vth, -vth),
    # mirroring Bass.__init__'s own const registration.
    for _v in (float(vth), float(-vth)):
        if (f32, _v) not in nc.const_aps.aps:
            _t = nc.alloc_sbuf_tensor(f"const-f32-{_v}", [128, 1], f32)
            nc.gpsimd.memset(_t.ap(), _v)
            nc.const_aps.aps[(f32, _v)] = _t.ap()
    nc.all_engine_barrier()

    BP = 2                       # batches per tile: amortizes DVE op overhead
    NS = _BS // BP
    FD = BP * _NB * _W           # per-step free-dim elements (784)
    with TileContext(nc) as tc:
        with (
            tc.tile_pool(name="xin", bufs=3) as xp,
            tc.tile_pool(name="out", bufs=3) as outp,
            tc.tile_pool(name="tmp", bufs=2) as tp,
        ):
            for s in range(NS):
                xt = xp.tile([_C, BP * HW], f32, tag="xt")
                nc.sync.dma_start(
                    out=xt[:, :].rearrange("c (b hw) -> c b hw", hw=HW),
                    in_=x[s * BP:(s + 1) * BP].rearrange("b c h w -> c b (h w)"),
                )
                ot = outp.tile([_C, BP * HW], f32, tag="ot")
                u = tp.tile([_C, FD], f32, tag="u")
                o = tp.tile([_C, FD], f32, tag="o")

                # (b nb) merge is valid: b stride = NB*LIF*W, nb stride =
                # LIF*W -> uniform [count BP*NB, stride LIF*W] + j*W offset.
                xv = xt[:, :].rearrange("c (q l w) -> c q l w", l=_LIF, w=_W)
                ov = ot[:, :].rearrange("c (q l w) -> c q l w", l=_LIF, w=_W)
                u3 = u[:, :].rearrange("c (q w) -> c q w", w=_W)
                o3 = o[:, :].rearrange("c (q w) -> c q w", w=_W)

                for j in range(_LIF):
                    xj = xv[:, :, j, :]
                    if j == 0:
                        uin = xj                       # u_0 = x_0 (o starts at 0)
                    else:
                        # u = (o * tau) + x_j
                        nc.vector.scalar_tensor_tensor(
                            out=u3, in0=o3, scalar=tau, in1=xj,
                            op0=op.mult, op1=op.add,
                        )
                        uin = u3
                    if j < _LIF - 1:
                        # o' = (u <= vth) * u
                        nc.vector.scalar_tensor_tensor(
                            out=o3, in0=uin, scalar=vth, in1=uin,
                            op0=op.is_le, op1=op.mult,
                        )
                    # out_j = max(u, vth). Engine split: DVE is the critical
                    # path, so most steps compute it on the otherwise-idle
                    # ScalarE as relu(u - vth) + vth (<=2 ulp off on spikes;
                    # the recurrence state o' stays exact on DVE).
                    oj = ov[:, :, j, :]
                    if j < 6:
                        r = tp.tile([_C, FD], f32, tag=f"r{j % 2}")
                        r3 = r[:, :].rearrange("c (q w) -> c q w", w=_W)
                        nc.scalar.activation(
                            out=r3, in_=uin,
                            func=mybir.ActivationFunctionType.Relu,
                            bias=-vth, scale=1.0,
                        )
                        nc.scalar.activation(
                            out=oj, in_=r3,
                            func=mybir.ActivationFunctionType.Identity,
                            bias=vth, scale=1.0,
                        )
                    else:
                        nc.vector.tensor_scalar_max(out=oj, in0=uin, scalar1=vth)
                nc.scalar.dma_start(
                    out=y[s * BP:(s + 1) * BP].rearrange("b c h w -> c b (h w)"),
                    in_=ot[:, :].rearrange("c (b hw) -> c b hw", hw=HW),
                )
    nc.finalize()   # runs Bacc.compile(): reg alloc + event-sem wait splitting
    return nc


def _get_nc(tau: float, vth: float):
    key = (tau, vth)
    if key not in _CACHE:
        _CACHE[key] = _build(tau, vth)
    return _CACHE[key]


def _run(x, tau, vth, **spmd_kwargs):
    from concourse.bass_utils import run_bass_kernel_spmd

    x = np.ascontiguousarray(np.asarray(x, dtype=np.float32))
    assert x.shape == (_B, _C, _H, _W), x.shape
    tau_f = float(np.asarray(tau).reshape(-1)[0])
    vth_f = float(np.asarray(vth).reshape(-1)[0])
    nc = _get_nc(tau_f, vth_f)
    shards = np.split(x, _NCORES, axis=0)
    in_maps = [{"x": np.ascontiguousarray(s)} for s in shards]
    res = run_bass_kernel_spmd(nc, in_maps, list(range(_NCORES)), **spmd_kwargs)
    out = np.concatenate([r["y"] for r in res.results], axis=0)
    return out, res


def kernel(x, tau, vth, lif, dim):
    assert int(np.asarray(lif)) == _LIF and int(np.asarray(dim)) == 2
    out, _ = _run(x, tau, vth)
    return out


# revision 17
# speedup vs baseline: 4.4155x; 1.0593x over previous
"""LIF spike kernel for Trainium2 (8 NeuronCores, batch-parallel).

Problem: x [64,128,56,56] f32; LIF recurrence in blocks of lif=8 steps along
H (dim=2):
    u   = tau*o + x_j
    out = u if u > vth else vth          == max(u, vth)
    o'  = 0 if u > vth else u            == (u <= vth) * u

Sharding: batch dim 64 -> 8 cores x 8 batches, no communication.
Per-core layout: C=128 on partitions, free dim = H*W per batch tile.
"""

import numpy as np

_B, _C, _H, _W = 64, 128, 56, 56
_NCORES = 8
_BS = _B // _NCORES          # batches per core
_LIF = 8
_NB = _H // _LIF             # 7 blocks per image

_CACHE = {}


def _build(tau: float, vth: float):
    import concourse.bacc as bacc
    import concourse.mybir as mybir
    from concourse.tile import TileContext

    f32 = mybir.dt.float32
    op = mybir.AluOpType

    # Bacc (not raw Bass): its compile() runs generate_event_semaphores,
    # which splits multi-waits into EventSemaphore instructions (TRN2 allows
    # only one sem wait per instruction).
    nc = bacc.Bacc("TRN2")
    x = nc.dram_tensor("x", [_BS, _C, _H, _W], f32, kind="ExternalInput")
    y = nc.dram_tensor("y", [_BS, _C, _H, _W], f32, kind="ExternalOutput")

    HW = _H * _W

    # Engine split per step j (recurrence along the lif axis):
    #   DVE:    u  = (o * tau) + x_j        [scalar_tensor_tensor]
    #           o' = (u <= vth) * u         [scalar_tensor_tensor]
    #   GPSIMD: out_j = max(u, vth)         [1-input op, ~line rate]
    # DMA-ins on the SP HWDGE ring, DMA-outs on the ACT ring so the two
    # streams don't serialize behind one descriptor FIFO.
    # Register const bias APs for the ScalarE activation path (+vth, -vth),
    # mirroring Bass.__init__'s own const registration.
    for _v in (float(vth), float(-vth)):
        if (f32, _v) not in nc.const_aps.aps:
            _t = nc.alloc_sbuf_tensor(f"const-f32-{_v}", [128, 1], f32)
            nc.gpsimd.memset(_t.ap(), _v)
            nc.const_aps.aps[(f32, _v)] = _t.ap()
    nc.all_engine_barrier()

    BP = 2                       # batches per tile: amortizes DVE op overhead
    NS = _BS // BP
    FD = BP * _NB * _W           # per-step free-dim elements (784)
    with TileContext(nc) as tc:
        with (
            tc.tile_pool(name="xin", bufs=3) as xp,
            tc.tile_pool(name="out", bufs=3) as outp,
            tc.tile_pool(name="tmp", bufs=2) as tp,
        ):
            for s in range(NS):
                xt = xp.tile([_C, BP * HW], f32, tag="xt")
                nc.sync.dma_start(
                    out=xt[:, :].rearrange("c (b hw) -> c b hw", hw=HW),
                    in_=x[s * BP:(s + 1) * BP].rearrange("b c h w -> c b (h w)"),
                )
                ot = outp.tile([_C, BP * HW], f32, tag="ot")
                o = tp.tile([_C, FD], f32, tag="o")

                # (b nb) merge is valid: b stride = NB*LIF*W, nb stride =
                # LIF*W -> uniform [count BP*NB, stride LIF*W] + j*W offset.
                xv = xt[:, :].rearrange("c (q l w) -> c q l w", l=_LIF, w=_W)
                ov = ot[:, :].rearrange("c (q l w) -> c q l w", l=_LIF, w=_W)
                o3 = o[:, :].rearrange("c (q w) -> c q w", w=_W)

                for j in range(_LIF):
                    xj = xv[:, :, j, :]
                    if j == 0:
                        uin = xj                       # u_0 = x_0 (o starts at 0)
                    else:
                        # u = (o * tau) + x_j. Fresh u slot per step so the
                        # trailing ScalarE reader never WAR-blocks this write.
                        u = tp.tile([_C, FD], f32, tag=f"u{j % 2}")
                        u3 = u[:, :].rearrange("c (q w) -> c q w", w=_W)
                        nc.vector.scalar_tensor_tensor(
                            out=u3, in0=o3, scalar=tau, in1=xj,
                            op0=op.mult, op1=op.add,
                        )
                        uin = u3
                    if j < _LIF - 1:
                        # o' = (u <= vth) * u
                        nc.vector.scalar_tensor_tensor(
                            out=o3, in0=uin, scalar=vth, in1=uin,
                            op0=op.is_le, op1=op.mult,
                        )
                    # out_j = max(u, vth). Engine split: DVE is the critical
                    # path, so most steps compute it on the otherwise-idle
                    # ScalarE as relu(u - vth) + vth (<=2 ulp off on spikes;
                    # the recurrence state o' stays exact on DVE).
                    oj = ov[:, :, j, :]
                    if j < 6:
                        r = tp.tile([_C, FD], f32, tag=f"r{j % 2}")
                        r3 = r[:, :].rearrange("c (q w) -> c q w", w=_W)
                        nc.scalar.activation(
                            out=r3, in_=uin,
                            func=mybir.ActivationFunctionType.Relu,
                            bias=-vth, scale=1.0,
                        )
                        nc.scalar.activation(
                            out=oj, in_=r3,
                            func=mybir.ActivationFunctionType.Identity,
                            bias=vth, scale=1.0,
                        )
                    else:
                        nc.vector.tensor_scalar_max(out=oj, in0=uin, scalar1=vth)
                nc.scalar.dma_start(
                    out=y[s * BP:(s + 1) * BP].rearrange("b c h w -> c b (h w)"),
                    in_=ot[:, :].rearrange("c (b hw) -> c b hw", hw=HW),
                )
    nc.finalize()   # runs Bacc.compile(): reg alloc + event-sem wait splitting
    return nc


def _get_nc(tau: float, vth: float):
    key = (tau, vth)
    if key not in _CACHE:
        _CACHE[key] = _build(tau, vth)
    return _CACHE[key]


def _run(x, tau, vth, **spmd_kwargs):
    from concourse.bass_utils import run_bass_kernel_spmd

    x = np.ascontiguousarray(np.asarray(x, dtype=np.float32))
    assert x.shape == (_B, _C, _H, _W), x.shape
    tau_f = float(np.asarray(tau).reshape(-1)[0])
    vth_f = float(np.asarray(vth).reshape(-1)[0])
    nc = _get_nc(tau_f, vth_f)
    shards = np.split(x, _NCORES, axis=0)
    in_maps = [{"x": np.ascontiguousarray(s)} for s in shards]
    res = run_bass_kernel_spmd(nc, in_maps, list(range(_NCORES)), **spmd_kwargs)
    out = np.concatenate([r["y"] for r in res.results], axis=0)
    return out, res


def kernel(x, tau, vth, lif, dim):
    assert int(np.asarray(lif)) == _LIF and int(np.asarray(dim)) == 2
    out, _ = _run(x, tau, vth)
    return out
